# revision 1
# baseline (speedup 1.0000x reference)
"""Distributed Bass kernel for nn_AttentionLayer (2-branch GAT-style layer).

Row-shard over 8 NeuronCores (512 rows each). All per-row tensors are kept
in "transposed" layout on chip (feature/column axis on SBUF partitions) so
that the masked softmax feeds the PE attention matmuls without transposes:

  e_b^T[k, i] = lrelu(s1_b[i] + s2_b[k])                (k on partitions)
  z = e + (mask01 - 1)*BIG ; p = exp(z)                 (exp underflow -> exact 0)
  out_b^T[f, i] = sum_k Wh_b[k, f] * p[k, i]            (PE, bf16)

adj2^T is computed on PE in fp8 DoubleRow (exact: adj is 0/1, psum f32):
  adj2^T[k, i] = sum_t adj_full[t, k] * adjT_shard[t, i]

The adj AllGather is split into 8 column chunks so branch-2 groups pipeline
against the collective. BatchNorm batch stats via a tiny AllReduce. No
row-max subtraction in softmax (values are small, no empty rows).
"""

import sys
import numpy as np

for _p in ("/opt/trn_rl_repo", "/opt/trn_rl_repo/concourse"):
    if _p not in sys.path:
        sys.path.insert(0, _p)

import ml_dtypes

N = 4096
M_CORES = 8
R = N // M_CORES          # 512 rows per core
IN_F = 512
HALF = IN_F // 2          # 256
F = 64
P = 128                   # partitions
NT = N // P               # 32 column tiles
NTP = NT // 2             # 16 row-pair tiles (DoubleRow)
RT = R // P               # 4 row tiles per core
G = 4                     # adj2 k-tiles per psum group
NG = NT // G              # 8 groups == adj AG chunks
ALPHA = 0.2
EPS = 1e-5
BIG = 9e15
INV_N = 1.0 / N

_CACHED = {}


def build_nc():
    from concourse import bacc, tile, mybir

    f32 = mybir.dt.float32
    bf16 = mybir.dt.bfloat16
    fp8 = mybir.dt.float8e4
    Alu = mybir.AluOpType
    Act = mybir.ActivationFunctionType
    DR = mybir.MatmulPerfMode.DoubleRow

    nc = bacc.Bacc("TRN2", target_bir_lowering=False, debug=False,
                   num_devices=M_CORES)

    hT_p = nc.declare_dram_parameter("hT", [IN_F, R], f32, isOutput=False)
    hTf_p = nc.declare_dram_parameter("hTf", [IN_F, N], f32, isOutput=False)
    adjb8_p = nc.declare_dram_parameter("adjb8", [R, N], fp8, isOutput=False)
    adjbT_p = nc.declare_dram_parameter("adjbT", [N, R], fp8, isOutput=False)
    dT_p = nc.declare_dram_parameter("dT", [N, R], bf16, isOutput=False)
    W_p = nc.declare_dram_parameter("W12", [HALF, 2 * F], f32, isOutput=False)
    a_p = nc.declare_dram_parameter("a", [F, 2], f32, isOutput=False)
    gb_p = nc.declare_dram_parameter("gb", [2 * F, 2], f32, isOutput=False)
    id_p = nc.declare_dram_parameter("ident", [P, P], f32, isOutput=False)
    out_p = nc.declare_dram_parameter("out", [R, 2 * F], f32, isOutput=True)

    RG = [list(range(M_CORES))]

    with tile.TileContext(nc) as tc:
        with (
            tc.tile_pool(name="sb", bufs=1) as sb,
            tc.tile_pool(name="sbt", bufs=3) as sbt,
            tc.tile_pool(name="psA", bufs=1, space="PSUM") as psA,
            tc.tile_pool(name="psB", bufs=5, space="PSUM") as psB,
            tc.tile_pool(name="dram", bufs=1, space="DRAM") as dram,
        ):
            # ---- small persistent loads (sync queue; gate the Wh compute)
            ident = sb.tile([P, P], f32)
            nc.sync.dma_start(ident[:], id_p[:])
            a_sb = sb.tile([F, 2], f32)
            nc.sync.dma_start(a_sb[:], a_p[:])
            gb_sb = sb.tile([2 * F, 2], f32)
            nc.sync.dma_start(gb_sb[:], gb_p[:])
            W_sb = []
            for t in range(2):
                w = sb.tile([P, 2 * F], f32, tag=f"w{t}")
                nc.sync.dma_start(w[:], W_p[P * t:P * (t + 1), :])
                W_sb.append(w)
            hT_sb = []
            for t in range(RT):
                ht = sb.tile([P, R], f32, tag=f"ht{t}")
                nc.sync.dma_start(ht[:], hT_p[P * t:P * (t + 1), :])
                hT_sb.append(ht)

            ones1 = sb.tile([1, P], f32)
            nc.vector.memset(ones1[:], 1.0)
            onesb = sb.tile([P, 1], bf16)
            nc.vector.memset(onesb[:], 1.0)

            # ---- adj fp8 chunk bounces for chunked AllGather (gpsimd queue)
            adj_in = []
            for c in range(NG):
                ai = dram.tile([R, R], fp8, name=f"adj_in{c}")
                nc.gpsimd.dma_start(ai[:], adjb8_p[:, R * c:R * (c + 1)])
                adj_in.append(ai)

            # ---- Wh^T = W^T @ h^T  (psum [128, 512]: b1 rows 0:64, b2 64:128)
            whT_ps = psA.tile([P, R], f32, tag="acc")
            for b in range(2):
                for t in range(2):
                    nc.tensor.matmul(
                        whT_ps[F * b:F * (b + 1), :],
                        W_sb[t][:, F * b:F * (b + 1)],
                        hT_sb[2 * b + t][:],
                        start=(t == 0), stop=(t == 1),
                    )
            whT_sb = sb.tile([P, R], f32)
            nc.vector.tensor_copy(whT_sb[:], whT_ps[:])
            # base-partition-0 copy of Wh2^T (PE shift via identity)
            wh2_ps = psB.tile([F, R], f32, tag="tmp")
            nc.tensor.matmul(wh2_ps[:], ident[F:P, F:P], whT_sb[F:P, :],
                             start=True, stop=True)
            whT2_sb = sb.tile([F, R], f32)
            nc.vector.tensor_copy(whT2_sb[:], wh2_ps[:])
            whT_b = [whT_sb, whT2_sb]

            # ---- s1 vectors (own rows): s1_b[i] = sum_f a1[f] * WhT_b[f, i]
            s1_sb = []
            for b in range(2):
                sv = psB.tile([1, R], f32, tag="tmp")
                nc.tensor.matmul(sv[:], a_sb[:, 0:1], whT_b[b][0:F, :],
                                 start=True, stop=True)
                dst = sb.tile([1, R], f32, tag=f"s1_{b}")
                nc.vector.tensor_copy(dst[:], sv[:])
                s1_sb.append(dst)

            # ---- collectives: only the 8 adj chunk AllGathers (+ final AR)
            adj_chunk = []
            for c in range(NG):
                ac = dram.tile([N, R], fp8, addr_space="Shared",
                               name=f"adj_chunk{c}")
                nc.gpsimd.collective_compute(
                    "AllGather", Alu.bypass, replica_groups=RG,
                    ins=[adj_in[c][:].opt()], outs=[ac[:].opt()])
                adj_chunk.append(ac)

            # ---- transposed adj shard (fp8, DoubleRow pairing) ----
            adjT_sb = []
            for t in range(NTP):
                at = sb.tile([P, 2, R], fp8, tag=f"adjT{t}")
                src = adjbT_p[2 * P * t:2 * P * (t + 1), :]
                nc.sync.dma_start(at[:], src.rearrange("(s p) i -> p s i",
                                                       p=P))
                adjT_sb.append(at)

            # ---- full Wh^T computed locally (no AllGather) ----
            whTf1 = sb.tile([F, N], f32)
            whTf2 = sb.tile([F, N], f32)
            whTf_b = [whTf1, whTf2]
            for ch in range(NG):
                hfs = []
                for t in range(RT):
                    hf = sbt.tile([P, R], f32, tag="hf", bufs=8)
                    nc.sync.dma_start(
                        hf[:], hTf_p[P * t:P * (t + 1), R * ch:R * (ch + 1)])
                    hfs.append(hf)
                for b in range(2):
                    wf_ps = psB.tile([F, R], f32, tag="tmp",
                                     name=f"wfps{ch}_{b}")
                    for t in range(2):
                        nc.tensor.matmul(
                            wf_ps[:],
                            W_sb[t][:, F * b:F * (b + 1)],
                            hfs[2 * b + t][:],
                            start=(t == 0), stop=(t == 1),
                        )
                    nc.vector.tensor_copy(
                        whTf_b[b][:, R * ch:R * (ch + 1)], wf_ps[:])

            # ---- s2 full, locally; spread to per-partition [p, kt] layout
            s2d = dram.tile([2, N], f32)
            for b in range(2):
                s2fl = sb.tile([1, N], f32, tag=f"s2fl{b}")
                for ch in range(NG):
                    sv = psB.tile([1, R], f32, tag="tmp")
                    nc.tensor.matmul(
                        sv[:], a_sb[:, 1:2],
                        whTf_b[b][:, R * ch:R * (ch + 1)],
                        start=True, stop=True)
                    nc.vector.tensor_copy(s2fl[:, R * ch:R * (ch + 1)], sv[:])
                nc.sync.dma_start(s2d[b:b + 1, :], s2fl[:])
            s2_sb = []
            for b in range(2):
                s2b = sb.tile([P, NT], f32, tag=f"s2_{b}")
                nc.sync.dma_start(s2b[:],
                                  s2d[b].rearrange("(kt p) -> p kt", p=P))
                s2_sb.append(s2b)

            # ---- Wh natural tiles [k, 2F] bf16 via local PE transpose ----
            whf_sb = []
            for t in range(NT):
                wf = sbt.tile([P, 2 * F], bf16, tag=f"whf{t}", bufs=1)
                for b in range(2):
                    tpw = psB.tile([P, F], f32, tag="tmp",
                                   name=f"tpw{t}_{b}")
                    nc.tensor.transpose(tpw[:],
                                        whTf_b[b][:, P * t:P * (t + 1)],
                                        ident[0:F, 0:F])
                    nc.vector.tensor_copy(wf[:, F * b:F * (b + 1)], tpw[:])
                whf_sb.append(wf)

            # ---- s1 broadcast across partitions (PE outer-product with ones)
            s1bc = []
            for b in range(2):
                bc = psB.tile([P, R], f32, tag="tmp")
                nc.tensor.matmul(bc[:], ones1[:], s1_sb[b][:],
                                 start=True, stop=True)
                s1b = sb.tile([P, R], f32, tag=f"s1bc{b}")
                nc.vector.tensor_copy(s1b[:], bc[:])
                s1bc.append(s1b)

            # ---- accumulators ----
            accT = psA.tile([P, R], f32, tag="acc")     # [0:64] b1, [64:128] b2
            sum_1 = psA.tile([1, R], f32, tag="sum1", name="sum_1")
            sum_2 = psA.tile([1, R], f32, tag="sum2", name="sum_2")
            sums = [sum_1, sum_2]

            def softmax_tile(b, kt, mask_done_ap):
                """mask_done_ap: f32 [P, R] with (mask01-1) in {-1, 0}."""
                u = sbt.tile([P, R], f32, tag="u")
                nc.scalar.activation(u[:], s1bc[b][:], Act.Identity,
                                     bias=s2_sb[b][:, kt:kt + 1])
                e = sbt.tile([P, R], f32, tag="e")
                nc.vector.scalar_tensor_tensor(
                    e[:], u[:], ALPHA, u[:], op0=Alu.mult, op1=Alu.max)
                z = sbt.tile([P, R], f32, tag="z")
                nc.vector.scalar_tensor_tensor(
                    z[:], mask_done_ap, BIG, e[:], op0=Alu.mult, op1=Alu.add)
                pt = sbt.tile([P, R], bf16, tag="pt", bufs=8)
                nc.scalar.activation(pt[:], z[:], Act.Exp)
                nc.tensor.matmul(sums[b][:], onesb[:], pt[:],
                                 start=(kt == 0), stop=(kt == NT - 1))
                nc.tensor.matmul(accT[F * b:F * (b + 1), :],
                                 whf_sb[kt][:, F * b:F * (b + 1)], pt[:],
                                 start=(kt == 0), stop=(kt == NT - 1))

            # ---- branches interleaved per adj chunk: branch-2 group g
            # (DoubleRow adj2 + softmax), then branch-1 tiles 4g..4g+3 ----
            for g in range(NG):
                cnts = [psB.tile([P, R], f32, tag="tmp", name=f"cnt{g}_{j}")
                        for j in range(G)]
                for t in range(NTP):
                    af = sbt.tile([P, 2, R], fp8, tag="af")
                    src = adj_chunk[g][2 * P * t:2 * P * (t + 1), :]
                    nc.sync.dma_start(af[:],
                                      src.rearrange("(s p) k -> p s k", p=P))
                    for j in range(G):
                        nc.tensor.matmul(cnts[j][:],
                                         af[:, :, P * j:P * (j + 1)],
                                         adjT_sb[t][:],
                                         perf_mode=DR,
                                         start=(t == 0), stop=(t == NTP - 1))
                for j in range(G):
                    kt = G * g + j
                    dt_t = sbt.tile([P, R], bf16, tag="dt")
                    nc.sync.dma_start(dt_t[:], dT_p[P * kt:P * (kt + 1), :])
                    m2 = sbt.tile([P, R], f32, tag="m")
                    nc.vector.tensor_scalar(m2[:], cnts[j][:], 1.0, -1.0,
                                            op0=Alu.min, op1=Alu.add)
                    nc.vector.tensor_tensor(m2[:], m2[:], dt_t[:],
                                            op=Alu.subtract)
                    softmax_tile(1, kt, m2[:])
                for j in range(G):
                    kt = G * g + j
                    m1 = sbt.tile([P, R], f32, tag="m")
                    nc.vector.tensor_scalar(m1[:],
                                            adjT_sb[kt // 2][:, kt % 2, :],
                                            -1.0, None, op0=Alu.add)
                    softmax_tile(0, kt, m1[:])

            # ---- epilogue: normalize, BN stats + AllReduce, BN, lrelu ----
            hpT = sb.tile([P, R], f32)
            for b in range(2):
                rc = sb.tile([1, R], f32, tag=f"rc{b}")
                nc.vector.reciprocal(rc[:], sums[b][:])
                bc = psB.tile([P, R], f32, tag="tmp")
                nc.tensor.matmul(bc[:], ones1[:], rc[:],
                                 start=True, stop=True)
                rb = sbt.tile([P, R], f32, tag="u")
                nc.vector.tensor_copy(rb[:], bc[:])
                nc.vector.tensor_mul(hpT[F * b:F * (b + 1), :],
                                     accT[F * b:F * (b + 1), :],
                                     rb[F * b:F * (b + 1), :])

            sx = sb.tile([2 * F, 2], f32)
            nc.vector.tensor_reduce(sx[:, 0:1], hpT[:],
                                    axis=mybir.AxisListType.X, op=Alu.add)
            scr = sbt.tile([P, R], bf16, tag="pt", bufs=8)
            nc.scalar.activation(scr[:], hpT[:], Act.Square,
                                 accum_out=sx[:, 1:2])
            stats_in = dram.tile([2 * F, 2], f32)
            nc.sync.dma_start(stats_in[:], sx[:])
            stats_out = dram.tile([2 * F, 2], f32, addr_space="Shared")
            nc.gpsimd.collective_compute(
                "AllReduce", Alu.add, replica_groups=RG,
                ins=[stats_in[:].opt()], outs=[stats_out[:].opt()])
            gst = sb.tile([2 * F, 2], f32)
            nc.sync.dma_start(gst[:], stats_out[:])

            mean = sb.tile([2 * F, 1], f32)
            nc.scalar.mul(mean[:], gst[:, 0:1], INV_N)
            ex2 = sb.tile([2 * F, 1], f32)
            nc.scalar.mul(ex2[:], gst[:, 1:2], INV_N)
            var = sb.tile([2 * F, 1], f32)
            nc.vector.scalar_tensor_tensor(var[:], mean[:], -1.0, mean[:],
                                           op0=Alu.mult, op1=Alu.mult)
            nc.vector.tensor_add(var[:], var[:], ex2[:])  # ex2 - mean^2
            nc.vector.tensor_scalar_add(var[:], var[:], EPS)
            std = sb.tile([2 * F, 1], f32)
            nc.scalar.activation(std[:], var[:], Act.Sqrt)
            rstd = sb.tile([2 * F, 1], f32)
            nc.vector.reciprocal(rstd[:], std[:])
            scale = sb.tile([2 * F, 1], f32)
            nc.vector.tensor_mul(scale[:], gb_sb[:, 0:1], rstd[:])
            nbias = sb.tile([2 * F, 1], f32)
            nc.vector.scalar_tensor_tensor(nbias[:], mean[:], -1.0, scale[:],
                                           op0=Alu.mult, op1=Alu.mult)
            nc.vector.tensor_add(nbias[:], nbias[:], gb_sb[:, 1:2])

            fin = sb.tile([P, R], f32)
            nc.scalar.activation(fin[:], hpT[:], Act.Identity,
                                 bias=nbias[:], scale=scale[:])
            finl = sb.tile([P, R], f32)
            nc.vector.scalar_tensor_tensor(finl[:], fin[:], ALPHA, fin[:],
                                           op0=Alu.mult, op1=Alu.max)

            for q in range(RT):
                tp = psB.tile([P, P], f32, tag="tmp")
                nc.tensor.transpose(tp[:], finl[:, P * q:P * (q + 1)],
                                    ident[:])
                ob = sbt.tile([P, P], f32, tag="ob")
                nc.vector.tensor_copy(ob[:], tp[:])
                nc.sync.dma_start(out_p[P * q:P * (q + 1), :], ob[:])

    nc.compile()
    return nc


def _get_nc():
    if "nc" not in _CACHED:
        _CACHED["nc"] = build_nc()
    return _CACHED["nc"]


def make_in_maps(h, adj, W1, W2, a, gamma, beta):
    h = np.asarray(h, dtype=np.float32)
    adj = np.asarray(adj, dtype=np.float32)
    W12 = np.concatenate([np.asarray(W1, np.float32),
                          np.asarray(W2, np.float32)], axis=1)
    a_flat = np.asarray(a, np.float32).reshape(2 * F)
    a_np = np.ascontiguousarray(np.stack([a_flat[:F], a_flat[F:]], axis=1))
    gb = np.stack([np.asarray(gamma, np.float32),
                   np.asarray(beta, np.float32)], axis=1)
    ident = np.eye(P, dtype=np.float32)

    adj_f8 = adj.astype(ml_dtypes.float8_e4m3fn)
    hTf = np.ascontiguousarray(h.T)

    in_maps = []
    for c in range(M_CORES):
        r0 = c * R
        sh = adj_f8[r0:r0 + R, :]
        dT = np.zeros((N, R), dtype=ml_dtypes.bfloat16)
        dT[np.arange(r0, r0 + R), np.arange(R)] = 1
        in_maps.append({
            "hT": np.ascontiguousarray(h[r0:r0 + R, :].T),
            "hTf": hTf,
            "adjb8": np.ascontiguousarray(sh),
            "adjbT": np.ascontiguousarray(sh.T),
            "dT": dT,
            "W12": W12,
            "a": a_np,
            "gb": gb,
            "ident": ident,
        })
    return in_maps


def kernel(h, adj, W1, W2, a, gamma, beta):
    from concourse.bass_utils import run_bass_kernel_spmd

    in_maps = make_in_maps(h, adj, W1, W2, a, gamma, beta)
    nc = _get_nc()
    res = run_bass_kernel_spmd(nc, in_maps, core_ids=list(range(M_CORES)))
    outs = [np.asarray(res.results[c]["out"]) for c in range(M_CORES)]
    return np.concatenate(outs, axis=0)



# revision 17
# speedup vs baseline: 1.3152x; 1.3152x over previous
"""Distributed Bass kernel for nn_AttentionLayer (2-branch GAT-style layer).

Row-shard over 8 NeuronCores (512 rows each), transposed on-chip layout
(k on partitions, own-row i on free axis) so masked softmax feeds the PE
attention matmuls without transposes.

Key structure (v2):
- Full adj is REPLICATED to every core in HBM (fp8, DoubleRow layout):
  no adj AllGather, no DRAM bounce copies; cores stream it at HBM rate.
- Wh is computed once per core for its own rows and AllGathered as a
  small [512, 130] bf16 payload (Wh1 | ones | Wh2 | ones); the ones
  columns make each attention matmul also produce the softmax row-sum
  (stationary [128, 65] -> psum [65, 512], row 64 = denominator).
- lrelu is fused into the activation instructions via Prelu (alpha=0.2),
  which shares the activation table set with Exp/Identity/Square (no
  table reloads).
- Masking via moderate-bias trick: p = exp(e + 40*mask01 - 40*diag - 40),
  with the -40 as the Exp activation bias.  Leakage exp(e-40) is ~1e-12.
- adj2 counts (2-hop) on PE in fp8 DoubleRow, exact in f32 psum.
- Attention matmuls run one group behind the adj2 matmuls (software
  pipelining) so the PE never waits on the Wh AllGather.
"""

import sys
import numpy as np

for _p in ("/opt/trn_rl_repo", "/opt/trn_rl_repo/concourse"):
    if _p not in sys.path:
        sys.path.insert(0, _p)

import ml_dtypes

N = 4096
M_CORES = 8
R = N // M_CORES          # 512 rows per core
IN_F = 512
HALF = IN_F // 2          # 256
F = 64
P = 128                   # partitions
NT = N // P               # 32 k tiles
NTP = NT // 2             # 16 DoubleRow k-tile pairs
G = 4                     # k-tiles per psum group
NG = NT // G              # 8 groups
ALPHA = 0.2
EPS = 1e-5
BIG2 = 40.0               # mask bias; exp(e - 40) ~ 0 for e <= ~12
INV_N = 1.0 / N

_CACHED = {}


def build_nc():
    from concourse import bacc, tile, mybir

    f32 = mybir.dt.float32
    bf16 = mybir.dt.bfloat16
    fp8 = mybir.dt.float8e4
    Alu = mybir.AluOpType
    Act = mybir.ActivationFunctionType
    DR = mybir.MatmulPerfMode.DoubleRow

    bf16_dt = mybir.dt.bfloat16
    nc = bacc.Bacc("TRN2", target_bir_lowering=False, debug=False,
                   num_devices=M_CORES)

    hT_p = nc.declare_dram_parameter("hT", [IN_F, R], f32, isOutput=False)
    adjdr_p = nc.declare_dram_parameter("adjdr", [NTP * P, 2, N], fp8,
                                        isOutput=False)
    adjT_p = nc.declare_dram_parameter("adjT", [NTP * P, 2, R], fp8,
                                       isOutput=False)
    dts_p = nc.declare_dram_parameter("dts", [N, R], bf16_dt, isOutput=False)
    W_p = nc.declare_dram_parameter("W12", [HALF, 2 * F], f32, isOutput=False)
    a4_p = nc.declare_dram_parameter("a4", [P, 4], f32, isOutput=False)
    gb_p = nc.declare_dram_parameter("gb", [F, 4], f32, isOutput=False)
    id_p = nc.declare_dram_parameter("ident", [P, P], f32, isOutput=False)
    out_p = nc.declare_dram_parameter("out", [R, 2 * F], f32, isOutput=True)

    RG = [list(range(M_CORES))]
    RQ = R // P               # 4 row blocks per core

    with tile.TileContext(nc) as tc:
        with (
            tc.tile_pool(name="sb", bufs=1) as sb,
            tc.tile_pool(name="aft", bufs=1) as aft,
            tc.tile_pool(name="sbt", bufs=3) as sbt,
            tc.tile_pool(name="psA", bufs=1, space="PSUM") as psA,
            tc.tile_pool(name="psC", bufs=5, space="PSUM") as psC,
            tc.tile_pool(name="psT", bufs=1, space="PSUM") as psT,
            tc.tile_pool(name="dram", bufs=1, space="DRAM") as dram,
        ):
            # ---- small persistent loads (sync queue) ----
            ident = sb.tile([P, P], f32)
            nc.sync.dma_start(ident[:], id_p[:])
            a4_sb = sb.tile([P, 4], f32)
            nc.sync.dma_start(a4_sb[:], a4_p[:])
            gb_sb = sb.tile([F, 4], f32)
            nc.sync.dma_start(gb_sb[:], gb_p[:])
            W_sb = []
            for t in range(2):
                w = sb.tile([P, 2 * F], f32, tag=f"w{t}")
                nc.sync.dma_start(w[:], W_p[P * t:P * (t + 1), :])
                W_sb.append(w)
            hT_sb = []
            for t in range(4):
                ht = sb.tile([P, R], f32, tag=f"ht{t}")
                nc.sync.dma_start(ht[:], hT_p[P * t:P * (t + 1), :])
                hT_sb.append(ht)
            ones1 = sb.tile([1, P], f32)
            nc.vector.memset(ones1[:], 1.0)
            neg40 = sb.tile([P, 1], f32)
            nc.vector.memset(neg40[:], -BIG2)
            # ones row at base partition 64 (for the sum-row broadcast)
            ones64 = sb.tile([F + 1, F], f32)
            nc.vector.memset(ones64[F:F + 1, :], 1.0)

            # ---- adjT shard (gpsimd queue, DR layout) ----
            adjT_sb = []
            for t in range(NTP):
                at = sb.tile([P, 2, R], fp8, tag=f"adjT{t}")
                nc.gpsimd.dma_start(at[:], adjT_p[P * t:P * (t + 1), :, :])
                adjT_sb.append(at)

            # ---- af tiles for group 0 (sync: even T, gpsimd: odd T) ----
            af_tiles = {}

            def load_af(g):
                for t in range(NTP):
                    af = aft.tile([P, 2, R], fp8, tag="af", bufs=48,
                                  name=f"af{g}_{t}")
                    q = nc.sync if t % 2 == 0 else nc.gpsimd
                    q.dma_start(af[:],
                                adjdr_p[P * t:P * (t + 1), :,
                                        R * g:R * (g + 1)])
                    af_tiles[(g, t)] = af

            load_af(0)

            # ---- preamble: own WhT, s-vectors, transpose to natural ----
            whT_ps = psT.tile([P, R], f32, tag="tmp", name="whT_ps")
            for b in range(2):
                for t in range(2):
                    nc.tensor.matmul(
                        whT_ps[F * b:F * (b + 1), :],
                        W_sb[t][:, F * b:F * (b + 1)],
                        hT_sb[2 * b + t][:],
                        start=(t == 0), stop=(t == 1),
                    )
            whT_sb = sb.tile([P, R], f32)
            nc.vector.tensor_copy(whT_sb[:], whT_ps[:])

            # s-vectors: 0/1 = s1_b1/s1_b2, 2/3 = s2_b1/s2_b2, each [1, R]
            # at base partition 0 (separate matmuls; PE requires base 0)
            scp = []
            for i in range(4):
                sv_ps = psC.tile([1, R], f32, tag="cnt", name=f"sv_ps{i}")
                nc.tensor.matmul(sv_ps[:], a4_sb[:, i:i + 1], whT_sb[:],
                                 start=True, stop=True)
                sc = sb.tile([1, R], f32, tag=f"scp{i}")
                nc.vector.tensor_copy(sc[:], sv_ps[:])
                scp.append(sc)

            # natural-layout own Wh (bf16) with ones cols:
            # cols 0:64 Wh1, 64 ones, 65:129 Wh2, 129 ones
            whfo_sb = sb.tile([P, RQ, 2 * F + 2], bf16)
            nc.vector.memset(whfo_sb[:, :, F:F + 1], 1.0)
            nc.vector.memset(whfo_sb[:, :, 2 * F + 1:2 * F + 2], 1.0)
            for q in range(RQ):
                tp = psT.tile([P, P], f32, tag="tmp", name=f"tpq{q}")
                nc.tensor.transpose(tp[:], whT_sb[:, P * q:P * (q + 1)],
                                    ident[:])
                nc.vector.tensor_copy(whfo_sb[:, q, 0:F], tp[:, 0:F])
                nc.vector.tensor_copy(whfo_sb[:, q, F + 1:2 * F + 1],
                                      tp[:, F:2 * F])

            # s1 broadcast across partitions (PE outer product)
            s1bc = []
            for b in range(2):
                bc = psC.tile([P, R], f32, tag="cnt", name=f"s1bc_ps{b}")
                nc.tensor.matmul(bc[:], ones1[:], scp[b][:],
                                 start=True, stop=True)
                s1b = sb.tile([P, R], f32, tag=f"s1bc{b}")
                nc.vector.tensor_copy(s1b[:], bc[:])
                s1bc.append(s1b)

            # ---- collectives: s2 first (gates softmax), then Wh ----
            s2o_d = dram.tile([1, 2, R], f32, name="s2o_d")
            for b in range(2):
                nc.sync.dma_start(s2o_d[0][b:b + 1, :], scp[2 + b][:])
            s2f_d = dram.tile([M_CORES, 2, R], f32, addr_space="Shared",
                              name="s2f_d")
            nc.gpsimd.collective_compute(
                "AllGather", Alu.bypass, replica_groups=RG,
                ins=[s2o_d[:].opt()], outs=[s2f_d[:].opt()])

            whfo_d = dram.tile([P, RQ, 2 * F + 2], bf16, name="whfo_d")
            nc.sync.dma_start(whfo_d[:], whfo_sb[:])
            whff_d = dram.tile([M_CORES * P, RQ, 2 * F + 2], bf16,
                               addr_space="Shared", name="whff_d")
            nc.gpsimd.collective_compute(
                "AllGather", Alu.bypass, replica_groups=RG,
                ins=[whfo_d[:].opt()], outs=[whff_d[:].opt()])

            # ---- AG-dependent loads (scalar queue); whf tiles load
            # lazily inside the main loop, 4 per group ----
            s2_sb = []
            for b in range(2):
                s2b = sb.tile([P, NT], f32, tag=f"s2_{b}")
                for c in range(M_CORES):
                    nc.scalar.dma_start(
                        s2b[:, RQ * c:RQ * (c + 1)],
                        s2f_d[c, b, :].rearrange("(q p) -> p q", p=P))
                s2_sb.append(s2b)

            whf_sb = [None] * NT

            def load_whf(g):
                for j in range(G):
                    kt = G * g + j
                    c, q = kt // RQ, kt % RQ
                    wt = sb.tile([P, 2 * F + 2], bf16, tag=f"whf{kt}")
                    nc.scalar.dma_start(wt[:],
                                        whff_d[P * c:P * (c + 1), q, :])
                    whf_sb[kt] = wt

            # ---- attention accumulators (psum rows 0:64 out, 64 sums) ----
            accT = [psA.tile([F + 1, R], f32, tag=f"acc{b}", name=f"accT{b}")
                    for b in range(2)]

            pt_b1 = {}
            pt_b2 = {}

            def softmax_b2(g, j, cnt):
                kt = G * g + j
                dt_t = sbt.tile([P, R], bf16, tag="dt", bufs=6)
                nc.scalar.dma_start(dt_t[:], dts_p[P * kt:P * (kt + 1), :])
                e2 = sbt.tile([P, R], f32, tag="e", bufs=4)
                nc.scalar.activation(e2[:], s1bc[1][:], Act.Prelu,
                                     bias=s2_sb[1][:, kt:kt + 1],
                                     alpha=ALPHA)
                m = sbt.tile([P, R], f32, tag="m", bufs=8)
                nc.vector.tensor_scalar(m[:], cnt[:], 1.0, BIG2,
                                        op0=Alu.min, op1=Alu.mult)
                nc.vector.tensor_tensor(m[:], m[:], dt_t[:], op=Alu.add)
                nc.vector.tensor_tensor(m[:], m[:], e2[:], op=Alu.add)
                pt = sbt.tile([P, R], bf16, tag="pt", bufs=18)
                nc.scalar.activation(pt[:], m[:], Act.Exp, bias=neg40[:])
                pt_b2[kt] = pt

            def softmax_b1(g, j):
                kt = G * g + j
                e1 = sbt.tile([P, R], f32, tag="e", bufs=4)
                nc.scalar.activation(e1[:], s1bc[0][:], Act.Prelu,
                                     bias=s2_sb[0][:, kt:kt + 1],
                                     alpha=ALPHA)
                z = sbt.tile([P, R], f32, tag="m", bufs=8)
                nc.vector.scalar_tensor_tensor(
                    z[:], adjT_sb[kt // 2][:, kt % 2, :], BIG2, e1[:],
                    op0=Alu.mult, op1=Alu.add)
                pt = sbt.tile([P, R], bf16, tag="pt", bufs=18)
                nc.scalar.activation(pt[:], z[:], Act.Exp, bias=neg40[:])
                pt_b1[kt] = pt

            def emit_att(g):
                for j in range(G):
                    kt = G * g + j
                    nc.tensor.matmul(accT[0][:],
                                     whf_sb[kt][:, 0:F + 1], pt_b1[kt][:],
                                     start=(kt == 0), stop=(kt == NT - 1))
                    nc.tensor.matmul(accT[1][:],
                                     whf_sb[kt][:, F + 1:2 * F + 2],
                                     pt_b2[kt][:],
                                     start=(kt == 0), stop=(kt == NT - 1))

            # ---- main loop: adj2 counts + softmax; att one group behind
            for g in range(NG):
                if g + 1 < NG:
                    load_af(g + 1)
                cnts = [psC.tile([P, R], f32, tag="cnt", name=f"cnt{g}_{j}")
                        for j in range(G)]
                for t in range(NTP):
                    af = af_tiles.pop((g, t))
                    for j in range(G):
                        nc.tensor.matmul(cnts[j][:],
                                         af[:, :, P * j:P * (j + 1)],
                                         adjT_sb[t][:],
                                         perf_mode=DR,
                                         start=(t == 0), stop=(t == NTP - 1))
                for j in range(G):
                    softmax_b2(g, j, cnts[j])
                for j in range(G):
                    softmax_b1(g, j)
                load_whf(g)
                if g >= 1:
                    emit_att(g - 1)
            emit_att(NG - 1)

            # ---- epilogue: normalize, BN stats + AllReduce, BN+lrelu ----
            # sums live at psum partition 64; keep all DVE ops aligned by
            # working in [65, R]-shaped tiles (row 64), then broadcasting
            # back to base-0 [64, R] per-branch tiles via PE.
            hp = []
            for b in range(2):
                srec = sb.tile([F + 1, R], f32, tag=f"srec{b}")
                nc.vector.tensor_copy(srec[F:F + 1, :], accT[b][F:F + 1, :])
                rrec = sb.tile([F + 1, R], f32, tag=f"rrec{b}")
                nc.vector.reciprocal(rrec[F:F + 1, :], srec[F:F + 1, :])
                bc_ps = psT.tile([F, R], f32, tag="tmp", name=f"bc_ps{b}")
                nc.tensor.matmul(bc_ps[:], ones64[F:F + 1, :],
                                 rrec[F:F + 1, :], start=True, stop=True)
                bc_sb = sb.tile([F, R], f32, tag=f"bcs{b}")
                nc.vector.tensor_copy(bc_sb[:], bc_ps[:])
                hp_b = sb.tile([F, R], f32, tag=f"hp{b}")
                nc.vector.tensor_tensor(hp_b[:], accT[b][0:F, :], bc_sb[:],
                                        op=Alu.mult)
                hp.append(hp_b)

            # stats packed [64, 4]: (sum1, sumsq1, sum2, sumsq2)
            sx = sb.tile([F, 4], f32)
            sq = sb.tile([F, R], bf16)
            for b in range(2):
                nc.vector.tensor_reduce(sx[:, 2 * b:2 * b + 1], hp[b][:],
                                        axis=mybir.AxisListType.X,
                                        op=Alu.add)
                nc.scalar.activation(sq[:], hp[b][:], Act.Square,
                                     accum_out=sx[:, 2 * b + 1:2 * b + 2])
            stats_in = dram.tile([F, 4], f32, name="stats_in")
            nc.sync.dma_start(stats_in[:], sx[:])
            stats_out = dram.tile([F, 4], f32, addr_space="Shared",
                                  name="stats_out")
            nc.gpsimd.collective_compute(
                "AllReduce", Alu.add, replica_groups=RG,
                ins=[stats_in[:].opt()], outs=[stats_out[:].opt()])
            gst = sb.tile([F, 4], f32)
            nc.sync.dma_start(gst[:], stats_out[:])

            # view [64, 2, 2]: [:, b, 0] = sum_b, [:, b, 1] = sumsq_b
            gst3 = gst[:].rearrange("f (b s) -> f b s", b=2)
            mean = sb.tile([F, 2], f32)
            nc.scalar.mul(mean[:], gst3[:, :, 0], INV_N)
            ex2 = sb.tile([F, 2], f32)
            nc.scalar.mul(ex2[:], gst3[:, :, 1], INV_N)
            var = sb.tile([F, 2], f32)
            nc.vector.scalar_tensor_tensor(var[:], mean[:], -1.0, mean[:],
                                           op0=Alu.mult, op1=Alu.mult)
            nc.vector.tensor_add(var[:], var[:], ex2[:])
            nc.vector.tensor_scalar_add(var[:], var[:], EPS)
            std = sb.tile([F, 2], f32)
            nc.scalar.activation(std[:], var[:], Act.Sqrt)
            rstd = sb.tile([F, 2], f32)
            nc.vector.reciprocal(rstd[:], std[:])
            # gamma/beta: gb rows 0:64 = branch1, 64:128 = branch2; repack
            # was done host-side into gb4 [64, 4] = (g1, b1, g2, b2)
            gb3 = gb_sb[:].rearrange("f (b s) -> f b s", b=2)
            scale = sb.tile([F, 2], f32)
            nc.vector.tensor_mul(scale[:], gb3[:, :, 0], rstd[:])
            nbias = sb.tile([F, 2], f32)
            nc.vector.scalar_tensor_tensor(nbias[:], mean[:], -1.0, scale[:],
                                           op0=Alu.mult, op1=Alu.mult)
            nc.vector.tensor_add(nbias[:], nbias[:], gb3[:, :, 1])

            # fused BN apply + lrelu; then transpose out per branch
            ob = sb.tile([P, RQ, 2 * F], f32)
            for b in range(2):
                finb = sb.tile([F, R], f32, tag=f"fin{b}")
                nc.scalar.activation(finb[:], hp[b][:], Act.Prelu,
                                     bias=nbias[:, b:b + 1],
                                     scale=scale[:, b:b + 1], alpha=ALPHA)
                for q in range(RQ):
                    tp = psT.tile([P, F], f32, tag="tmp", name=f"otp{b}_{q}")
                    nc.tensor.transpose(tp[:], finb[:, P * q:P * (q + 1)],
                                        ident[0:F, 0:F])
                    nc.vector.tensor_copy(ob[:, q, F * b:F * (b + 1)],
                                          tp[:])
            nc.sync.dma_start(
                out_p.rearrange("(q p) f -> p q f", p=P), ob[:])

    nc.compile()
    return nc


def _get_nc():
    if "nc" not in _CACHED:
        _CACHED["nc"] = build_nc()
    return _CACHED["nc"]


def make_in_maps(h, adj, W1, W2, a, gamma, beta):
    h = np.asarray(h, dtype=np.float32)
    adj = np.asarray(adj, dtype=np.float32)
    W12 = np.concatenate([np.asarray(W1, np.float32),
                          np.asarray(W2, np.float32)], axis=1)
    a_flat = np.asarray(a, np.float32).reshape(2 * F)
    a4 = np.zeros((P, 4), dtype=np.float32)
    a4[0:F, 0] = a_flat[:F]
    a4[F:2 * F, 1] = a_flat[:F]
    a4[0:F, 2] = a_flat[F:]
    a4[F:2 * F, 3] = a_flat[F:]
    gamma = np.asarray(gamma, np.float32)
    beta = np.asarray(beta, np.float32)
    # [64, 4] = (gamma1, beta1, gamma2, beta2) per feature
    gb = np.stack([gamma[:F], beta[:F], gamma[F:], beta[F:]], axis=1)
    ident = np.eye(P, dtype=np.float32)

    fp8 = ml_dtypes.float8_e4m3fn
    adj_f8 = adj.astype(fp8)
    # full adj in DoubleRow layout: adjdr[128T+p, s, k] = adj[256T+128s+p, k]
    adjdr = np.ascontiguousarray(
        adj_f8.reshape(NTP, 2, P, N).transpose(0, 2, 1, 3)
        .reshape(NTP * P, 2, N))

    in_maps = []
    for c in range(M_CORES):
        r0 = c * R
        shT = np.ascontiguousarray(adj[r0:r0 + R, :].T).astype(fp8)
        adjT = np.ascontiguousarray(
            shT.reshape(NTP, 2, P, R).transpose(0, 2, 1, 3)
            .reshape(NTP * P, 2, R))
        dts = np.zeros((N, R), dtype=ml_dtypes.bfloat16)
        dts[np.arange(r0, r0 + R), np.arange(R)] = -BIG2
        in_maps.append({
            "hT": np.ascontiguousarray(h[r0:r0 + R, :].T),
            "adjdr": adjdr,
            "adjT": adjT,
            "dts": dts,
            "W12": W12,
            "a4": a4,
            "gb": gb,
            "ident": ident,
        })
    return in_maps


def kernel(h, adj, W1, W2, a, gamma, beta):
    from concourse.bass_utils import run_bass_kernel_spmd

    in_maps = make_in_maps(h, adj, W1, W2, a, gamma, beta)
    nc = _get_nc()
    res = run_bass_kernel_spmd(nc, in_maps, core_ids=list(range(M_CORES)))
    outs = [np.asarray(res.results[c]["out"]) for c in range(M_CORES)]
    return np.concatenate(outs, axis=0)
